# revision 3
# baseline (speedup 1.0000x reference)
# DeepGEMM-style fp8 block-quantized linear for Trainium2, 8-core SPMD.
#
# reference semantics (see problem):
#   x_dq = dequant(quant_e4m3fn(x, per-token per-128-group amax/448 scales))
#   w_dq = w_fp8 * w_scale (per 128x128 block)
#   out  = (x_dq @ w_dq.T).astype(bf16)        # fp32 accumulation
#
# Device strategy (per core, 2x4 [M x N] grid => each core: M2=2048, N2=1024):
#   - TRN fp8_e4m3 tops out at 240 (vs OCP e4m3fn's 448), so quantize
#     y/4 = x * (112/amax) on TRN's grid: identical rounding for normals
#     (pure exponent shift), then dequant with s4 = amax/112.
#   - scales folded into fp16 matmul operands; bf16-exact weight cast via
#     gpsimd cast-DMA; on-chip xbar DMA transposes into [K,*] layouts.
#   - 128x512 psum tiles accumulate over the 56 k-blocks.

import numpy as np
import ml_dtypes
from contextlib import ExitStack

import concourse.bass as bass
import concourse.mybir as mybir
import concourse.tile as tile
from concourse import bacc
from concourse.bass_utils import run_bass_kernel_spmd

dt = mybir.dt

M, N, K = 4096, 4096, 7168
MSH, NSH = 2, 4                     # core grid: 2 along M, 4 along N
NCORES = MSH * NSH
BLK = 128


def emit_kernel(ctx, tc, o_d, x_d, w_d, ws_d):
    """One core's program. x:[M2,K]bf16, w:[N2,K]f32(fp8 vals), ws:[N2/128,K/128]f32 -> o:[M2,N2]bf16."""
    nc = tc.nc
    f32, f16, f8 = dt.float32, dt.float16, dt.float8e4
    bf16 = dt.bfloat16
    M2, Kd = x_d.shape
    N2, _ = w_d.shape
    KB = Kd // BLK            # k-blocks
    NB = N2 // BLK            # n-blocks
    MT = M2 // BLK            # m-tiles
    NH = N2 // 512            # psum halves per m-tile
    KC = 8                    # k-blocks per transpose chunk
    assert KB % KC == 0

    consts = ctx.enter_context(tc.tile_pool(name="consts", bufs=1))
    wtp = ctx.enter_context(tc.tile_pool(name="wt", bufs=1))
    natp = ctx.enter_context(tc.tile_pool(name="nat", bufs=2))
    scp = ctx.enter_context(tc.tile_pool(name="scales", bufs=2))
    xqp = ctx.enter_context(tc.tile_pool(name="xq", bufs=3))
    xdqp = ctx.enter_context(tc.tile_pool(name="xdq", bufs=3))
    xtp = ctx.enter_context(tc.tile_pool(name="xt", bufs=2))
    obp = ctx.enter_context(tc.tile_pool(name="ob", bufs=2))
    psp = ctx.enter_context(tc.tile_pool(name="ps", bufs=4, space="PSUM"))
    psbp = ctx.enter_context(tc.tile_pool(name="psb", bufs=1, space="PSUM"))

    # --- stage 0: broadcast w_scale across partitions: wsb[p, nb*KB+kb] = ws[nb, kb]
    ones = consts.tile([1, 128], f32)
    nc.vector.memset(ones[:], 1.0)
    wsflat = consts.tile([1, NB * KB], f32)
    nc.sync.dma_start(wsflat[:], ws_d[:, :])
    psb = psbp.tile([128, NB * KB], f32)
    nc.tensor.matmul(psb[:], ones[:], wsflat[:], start=True, stop=True)
    wsb = consts.tile([128, NB * KB], f32)
    nc.vector.tensor_copy(wsb[:], psb[:])

    # --- stage 1: weight prep. W_T[p, kb, n] = w[n, kb*128+p] * ws[n//128, kb]
    wt_t = wtp.tile([128, KB, N2], f16)
    for nb in range(NB):
        wq = natp.tile([128, Kd], f16, tag="nat")
        # f32 carrier -> fp16 is exact for e4m3fn values (cast during DMA)
        nc.gpsimd.dma_start(wq[:], w_d[nb * BLK:(nb + 1) * BLK, :])
        nc.sync.dma_start(wt_t[:, :, nb * BLK:(nb + 1) * BLK], wq[:], transpose=True)
        for kb in range(KB):
            sl = wt_t[:, kb, nb * BLK:(nb + 1) * BLK]
            nc.scalar.mul(sl, sl, wsb[:, nb * KB + kb: nb * KB + kb + 1])

    # --- stage 2: per m-tile: quantize, transpose, matmul
    for mt in range(MT):
        xn = natp.tile([128, Kd], bf16, tag="nat")
        nc.sync.dma_start(xn[:], x_d[mt * BLK:(mt + 1) * BLK, :])

        amax = scp.tile([128, KB], f32, tag="amax")
        nc.vector.reduce_max(
            amax[:],
            xn[:].rearrange("p (kb c) -> p kb c", c=BLK),
            axis=mybir.AxisListType.X,
            apply_absolute_value=True,
        )
        # s4 ~= max(amax, 1e-12) / 112  (== 4 * reference scale up to 1 ulp;
        # divide is not a valid DVE tensor_scalar op, so clamp+mul then an
        # exactly-rounded reciprocal)
        s4 = scp.tile([128, KB], f32, tag="s4")
        nc.vector.tensor_scalar(
            out=s4[:], in0=amax[:],
            scalar1=1e-12, scalar2=float(np.float32(1.0 / 112.0)),
            op0=mybir.AluOpType.max, op1=mybir.AluOpType.mult,
        )
        inv4 = scp.tile([128, KB], f32, tag="inv4")
        nc.vector.reciprocal(inv4[:], s4[:])

        xt_t = xtp.tile([128, KB, 128], f16, tag="xt")
        for c in range(KB // KC):
            xq = xqp.tile([128, KC * BLK], f8, tag="xq")
            xdq = xdqp.tile([128, KC * BLK], f16, tag="xdq")
            for j in range(KC):
                kb = c * KC + j
                nc.scalar.activation(
                    xq[:, j * BLK:(j + 1) * BLK],
                    xn[:, kb * BLK:(kb + 1) * BLK],
                    mybir.ActivationFunctionType.Copy,
                    scale=inv4[:, kb:kb + 1],
                )
                nc.vector.tensor_scalar_mul(
                    xdq[:, j * BLK:(j + 1) * BLK],
                    xq[:, j * BLK:(j + 1) * BLK],
                    s4[:, kb:kb + 1],
                )
            nc.sync.dma_start(xt_t[:, c * KC:(c + 1) * KC, :], xdq[:], transpose=True)

        ob = obp.tile([128, N2], bf16, tag="ob")
        for h in range(NH):
            pst = psp.tile([128, 512], f32, tag="ps")
            for kb in range(KB):
                nc.tensor.matmul(
                    pst[:],
                    xt_t[:, kb, :],
                    wt_t[:, kb, h * 512:(h + 1) * 512],
                    start=(kb == 0), stop=(kb == KB - 1),
                )
            nc.vector.tensor_copy(ob[:, h * 512:(h + 1) * 512], pst[:])
        nc.sync.dma_start(o_d[mt * BLK:(mt + 1) * BLK, :], ob[:])


def build_nc(m2, n2, k):
    nc = bacc.Bacc("TRN2", target_bir_lowering=False, debug=False, num_devices=NCORES)
    x_d = nc.dram_tensor("x", [m2, k], dt.bfloat16, kind="ExternalInput").ap()
    w_d = nc.dram_tensor("w", [n2, k], dt.float32, kind="ExternalInput").ap()
    ws_d = nc.dram_tensor("ws", [n2 // BLK, k // BLK], dt.float32, kind="ExternalInput").ap()
    o_d = nc.dram_tensor("o", [m2, n2], dt.bfloat16, kind="ExternalOutput").ap()
    with tile.TileContext(nc) as tc, ExitStack() as ctx:
        emit_kernel(ctx, tc, o_d, x_d, w_d, ws_d)
    nc.compile()
    return nc


_cache = {}


def _get_nc():
    if "nc" not in _cache:
        _cache["nc"] = build_nc(M // MSH, N // NSH, K)
    return _cache["nc"]


def kernel(input, weight_fp8, weight_scale, _trace=False, _trace_kwargs=None):
    input = np.asarray(input)
    if input.dtype != ml_dtypes.bfloat16:
        input = input.astype(ml_dtypes.bfloat16)
    weight_fp8 = np.asarray(weight_fp8, dtype=np.float32)
    weight_scale = np.asarray(weight_scale, dtype=np.float32)
    M2, N2 = M // MSH, N // NSH
    NSB, KSB = N2 // BLK, K // BLK

    in_maps = []
    for c in range(NCORES):
        mi, ni = divmod(c, NSH)
        in_maps.append({
            "x": np.ascontiguousarray(input[mi * M2:(mi + 1) * M2]),
            "w": np.ascontiguousarray(weight_fp8[ni * N2:(ni + 1) * N2]),
            "ws": np.ascontiguousarray(weight_scale[ni * NSB:(ni + 1) * NSB]),
        })

    nc = _get_nc()
    kw = {}
    if _trace:
        kw = dict(trace=True, **(_trace_kwargs or {}))
    res = run_bass_kernel_spmd(nc, in_maps, core_ids=list(range(NCORES)), **kw)

    out = np.empty((M, N), dtype=ml_dtypes.bfloat16)
    for c in range(NCORES):
        mi, ni = divmod(c, NSH)
        out[mi * M2:(mi + 1) * M2, ni * N2:(ni + 1) * N2] = res.results[c]["o"]
    if _trace:
        return out, res
    return out
